# revision 14
# baseline (speedup 1.0000x reference)
"""Trainium2 Bass kernel for CustomGRU (B=64, T=512, D=512, U=1024).

Sharding: data-parallel over batch across 8 NeuronCores (8 rows each),
weights replicated. Everything runs in the TRANSPOSED (U-major) layout:
the hidden state lives as hT[u, b] tiles [128, (uc, b)] = [128, 64], so
every gate matmul has the WEIGHT chunk [128, 128] as the stationary and
an 8-wide hT / xT chunk as the moving tensor (out [128, 8] PSUM column
slices). The input projection W^T x_t is folded into the recurrence as
4 extra contraction chunks per gate (no phase-1 GEMM, no xz staging in
DRAM, no per-step input DMA), and the bias is preloaded with a single
[K=8] matmul per gate (lhsT = bias reshaped [8, 128], moving =
eye8 kron ones8).

Per step t (per core):
  PE : r-bias/x/h (97 mm) -> c-bias/x (33) -> z-bias/x/h (97) -> c-rh (64)
  Act: rs=sigmoid(ps_r) [f16], zs=sigmoid(ps_z) [f32], hh=tanh(ps_c)
  DVE: rh=rs*hT, w=1-zs, zh=zs*h_prev, m1=w*hh, hT'=m1+zh [f16],
       h=m1+zh [f32 -> 16-step history ring, DMA-flushed to DRAM]

All matmul moving operands are f16 (1 cyc/row); weights/x are f16,
combine math f32. Host assembles [128, T, 8, 8] -> [B, T, U].
"""
import sys

if "/opt/trn_rl_repo" not in sys.path:
    sys.path.insert(0, "/opt/trn_rl_repo")

import numpy as np
from contextlib import ExitStack

import concourse.bass as bass
import concourse.bacc as bacc
import concourse.tile as tile
from concourse import mybir
from concourse.bass_utils import run_bass_kernel_spmd

F32 = mybir.dt.float32
F16 = mybir.dt.float16
MULT = mybir.AluOpType.mult
ADD = mybir.AluOpType.add
SIG = mybir.ActivationFunctionType.Sigmoid
TANH = mybir.ActivationFunctionType.Tanh

N_CORES = 8
B = 64
BS = B // N_CORES  # 8 batch rows per core
D = 512
U = 1024
KC = U // 128      # 8 contraction chunks over U
DC = D // 128      # 4 contraction chunks over D
NUC = U // 128     # 8 output u-chunks
FLUSH = 32         # steps per output-DMA flush


def build(nc, T, reps=1):
    # ---- DRAM I/O (per-core; weights replicated, x sharded) ----
    uz_d = nc.dram_tensor("uz", [128, KC * U], F16, kind="ExternalInput")
    ur_d = nc.dram_tensor("ur", [128, KC * U], F16, kind="ExternalInput")
    uh_d = nc.dram_tensor("uh", [128, KC * U], F16, kind="ExternalInput")
    wz_d = nc.dram_tensor("wz", [128, DC * U], F16, kind="ExternalInput")
    wr_d = nc.dram_tensor("wr", [128, DC * U], F16, kind="ExternalInput")
    wh_d = nc.dram_tensor("wh", [128, DC * U], F16, kind="ExternalInput")
    xt_d = nc.dram_tensor("xt", [128, DC * T * BS], F16, kind="ExternalInput")
    b8_d = nc.dram_tensor("b8", [8, 3 * 128], F16, kind="ExternalInput")
    e8_d = nc.dram_tensor("e8", [8, NUC * BS], F16, kind="ExternalInput")
    out_d = nc.dram_tensor("out", [128, T * 64], F32, kind="ExternalOutput")

    with tile.TileContext(nc) as tc, ExitStack() as ctx:
        const = ctx.enter_context(tc.tile_pool(name="const", bufs=1))
        uz = const.tile([128, KC * U], F16)
        nc.sync.dma_start(uz[:], uz_d[:])
        ur = const.tile([128, KC * U], F16)
        nc.sync.dma_start(ur[:], ur_d[:])
        uh = const.tile([128, KC * U], F16)
        nc.sync.dma_start(uh[:], uh_d[:])
        wz = const.tile([128, DC * U], F16)
        nc.sync.dma_start(wz[:], wz_d[:])
        wr = const.tile([128, DC * U], F16)
        nc.sync.dma_start(wr[:], wr_d[:])
        wh = const.tile([128, DC * U], F16)
        nc.sync.dma_start(wh[:], wh_d[:])
        xt = const.tile([128, DC * T * BS], F16)
        nc.sync.dma_start(xt[:], xt_d[:])
        b8 = const.tile([8, 3 * 128], F16)
        nc.sync.dma_start(b8[:], b8_d[:])
        e8 = const.tile([8, NUC * BS], F16)
        nc.sync.dma_start(e8[:], e8_d[:])
        hT0 = const.tile([128, 64], F16)
        nc.vector.memset(hT0[:], 0.0)
        h00 = const.tile([128, 64], F32)
        nc.vector.memset(h00[:], 0.0)

        hpool = ctx.enter_context(tc.tile_pool(name="h", bufs=2))
        sp = ctx.enter_context(tc.tile_pool(name="s", bufs=2))
        histp = ctx.enter_context(tc.tile_pool(name="hist", bufs=2))
        psum = ctx.enter_context(tc.tile_pool(name="ps", bufs=2, space="PSUM"))

        def xgate(ps, wt, g):
            # bias preload (start=True clears the whole [128,64] tile), then
            # W^T x_t: 4 d-chunks x 8 u-chunks of out [128, 8]
            nc.tensor.matmul(ps[:], b8[:, g * 128:(g + 1) * 128], e8[:],
                             start=True, stop=False, skip_group_check=True)
            for dc in range(DC):
                xs = xt[:, dc * (T * BS) + t * BS: dc * (T * BS) + (t + 1) * BS]
                for uc in range(NUC):
                    nc.tensor.matmul(
                        ps[:, uc * 8:(uc + 1) * 8],
                        wt[:, dc * U + uc * 128: dc * U + (uc + 1) * 128],
                        xs, start=False, stop=False, skip_group_check=True)

        def hgate(ps, ut, mov, stop=True, ks=0, ke=KC):
            # U^T h: k-chunks x 8 u-chunks; stop on the last write of
            # each column slice
            for k in range(ks, ke):
                ms = mov[:, k * 8:(k + 1) * 8]
                for uc in range(NUC):
                    nc.tensor.matmul(
                        ps[:, uc * 8:(uc + 1) * 8],
                        ut[:, k * U + uc * 128: k * U + (uc + 1) * 128],
                        ms, start=False, stop=(stop and k == KC - 1),
                        skip_group_check=True)

        def hgate_ucouter(ps, ut, mov):
            # uc-outer ordering: column slice uc is COMPLETE after its own
            # k-loop, so downstream column-half consumers can start early
            for uc in range(NUC):
                for k in range(KC):
                    nc.tensor.matmul(
                        ps[:, uc * 8:(uc + 1) * 8],
                        ut[:, k * U + uc * 128: k * U + (uc + 1) * 128],
                        mov[:, k * 8:(k + 1) * 8],
                        start=False, stop=(k == KC - 1),
                        skip_group_check=True)

        # r-gate consumes h = m1 + zh as two separate moving tensors, so
        # the critical path runs tanh -> m1 -> r-matmuls without waiting
        # for the materialized hT (which is produced off-path for z/rh).
        m1_prev = hT0
        zh_prev = hT0
        hT_prev = hT0
        hprev = h00[:]
        hist = None
        for rep in range(reps):
          for t in range(T):
            if t % FLUSH == 0:
                hist = histp.tile([128, FLUSH * 64], F32, tag="hist")
            ps_r = psum.tile([128, 64], F32, tag="r")
            ps_z = psum.tile([128, 64], F32, tag="z")
            ps_c = psum.tile([128, 64], F32, tag="c")

            # PE stream: r first (critical path: zh-part pre-runs, m1-part
            # is the only h-dependent leg and consumes m1 in halves), then
            # c/z fill the gaps
            xgate(ps_r, wr, 1)
            hgate(ps_r, ur, zh_prev[:], stop=False)
            hgate(ps_r, ur, m1_prev[:])
            xgate(ps_c, wh, 2)

            rs = sp.tile([128, 64], F16, tag="rs")
            nc.scalar.activation(rs[:], ps_r[:], SIG)
            rh = sp.tile([128, 64], F16, tag="rh")
            nc.vector.tensor_mul(rh[:], rs[:], hT_prev[:])

            xgate(ps_z, wz, 0)
            hgate(ps_z, uz, hT_prev[:])
            hgate_ucouter(ps_c, uh, rh[:])

            zs = sp.tile([128, 64], F32, tag="zs")
            nc.scalar.activation(zs[:], ps_z[:], SIG)
            w = sp.tile([128, 64], F16, tag="w")
            nc.vector.tensor_scalar(w[:], zs[:], -1.0, 1.0, MULT, ADD)
            zh = sp.tile([128, 64], F16, tag="zh")
            nc.vector.tensor_mul(zh[:], zs[:], hprev)

            hh = sp.tile([128, 64], F16, tag="hh")
            nc.scalar.activation(hh[:], ps_c[:], TANH)
            m1 = sp.tile([128, 64], F16, tag="m1")
            nc.vector.tensor_mul(m1[:], w[:], hh[:])
            hT_new = hpool.tile([128, 64], F16, tag="hT")
            nc.vector.tensor_add(hT_new[:], m1[:], zh[:])
            hs = hist[:, (t % FLUSH) * 64: (t % FLUSH + 1) * 64]
            nc.gpsimd.tensor_add(hs, m1[:], zh[:])
            if t % FLUSH == FLUSH - 1 or t == T - 1:
                base = t - (t % FLUSH)
                nc.sync.dma_start(
                    out_d[:, base * 64: (t + 1) * 64],
                    hist[:, : (t % FLUSH + 1) * 64])

            m1_prev = m1
            zh_prev = zh
            hT_prev = hT_new
            hprev = hs

    nc.compile()
    return nc


def _u_layout(M):
    # [U, U] -> [128, KC*U]: out[p, k*U+u] = M[k*128+p, u]
    return np.ascontiguousarray(
        np.asarray(M, np.float32).reshape(KC, 128, U).transpose(1, 0, 2)
        .reshape(128, KC * U)).astype(np.float16)


def _w_layout(M):
    # [D, U] -> [128, DC*U]: out[p, dc*U+u] = M[dc*128+p, u]
    return np.ascontiguousarray(
        np.asarray(M, np.float32).reshape(DC, 128, U).transpose(1, 0, 2)
        .reshape(128, DC * U)).astype(np.float16)


def prepare(inputs, Wz, Uz, bz, Wr, Ur, br, Wh, Uh, bh, T):
    """Build the Bass program and the per-core input maps."""
    x = np.asarray(inputs, dtype=np.float32)[:, :T, :]

    uz, ur, uh = _u_layout(Uz), _u_layout(Ur), _u_layout(Uh)
    wz, wr, wh = _w_layout(Wz), _w_layout(Wr), _w_layout(Wh)
    b8 = np.concatenate(
        [np.asarray(v, np.float32).reshape(8, 128) for v in (bz, br, bh)],
        axis=1).astype(np.float16)
    e8 = np.kron(np.eye(8, dtype=np.float16), np.ones((1, 8), np.float16))
    e8 = np.ascontiguousarray(e8)

    nc = bacc.Bacc("TRN2", target_bir_lowering=False, debug=False,
                   num_devices=N_CORES)
    build(nc, T)

    in_maps = []
    for c in range(N_CORES):
        xc = x[c * BS:(c + 1) * BS]               # [BS, T, D]
        # xt[p, dc, t, b] = xc[b, t, dc*128+p]
        xtc = np.ascontiguousarray(
            xc.reshape(BS, T, DC, 128).transpose(3, 2, 1, 0)
            .reshape(128, DC * T * BS)).astype(np.float16)
        in_maps.append({
            "uz": uz, "ur": ur, "uh": uh, "wz": wz, "wr": wr, "wh": wh,
            "xt": xtc, "b8": b8, "e8": e8,
        })
    return nc, in_maps


def assemble(results):
    outs = []
    T = results[0]["out"].shape[1] // 64
    for c in range(N_CORES):
        o = results[c]["out"]                     # [128, T*64]
        # o[p, t*64 + uc*8 + b] = h_t[b, uc*128+p]
        o = o.reshape(128, T, NUC, BS).transpose(3, 1, 2, 0)  # [b, t, uc, p]
        outs.append(np.ascontiguousarray(o.reshape(BS, T, U)))
    return np.concatenate(outs, axis=0)           # [B, T, U]


def kernel(inputs, Wz, Uz, bz, Wr, Ur, br, Wh, Uh, bh, _T=None):
    T = inputs.shape[1] if _T is None else _T
    nc, in_maps = prepare(inputs, Wz, Uz, bz, Wr, Ur, br, Wh, Uh, bh, T)
    res = run_bass_kernel_spmd(nc, in_maps, list(range(N_CORES)))
    return assemble(res.results)


# revision 22
# speedup vs baseline: 1.0786x; 1.0786x over previous
"""Trainium2 Bass kernel for CustomGRU (B=64, T=512, D=512, U=1024).

Sharding: data-parallel over batch across 8 NeuronCores (8 rows each),
weights replicated. Everything runs in the TRANSPOSED (U-major) layout:
the hidden state lives as hT[u, b] tiles [128, (uc, b)] = [128, 64], so
every gate matmul has the WEIGHT chunk [128, 128] as the stationary and
an 8-wide hT / xT chunk as the moving tensor (out [128, 8] PSUM column
slices). The input projection W^T x_t is folded into the recurrence as
4 extra contraction chunks per gate (no phase-1 GEMM, no xz staging in
DRAM, no per-step input DMA), and the bias is preloaded with a single
[K=8] matmul per gate (lhsT = bias reshaped [8, 128], moving =
eye8 kron ones8).

The per-step critical chain is kept to four cross-engine hops:
  sigmoid(ps_r) -> rh = rs*hT (DVE) -> c-rh matmuls (PE) ->
  tanh(ps_c)    -> m1 = (1-z)*hh (DVE) -> next step's r-gate matmuls
by (a) feeding the r-gate h = m1 + zh as TWO separate moving tensors
(the zh part and all bias/x parts pre-run mid-step; hT itself is
materialized off-path for the z-gate / rh), and (b) computing both z
and 1-z as two sigmoids straight off the z PSUM (scale=-1) on the Act
engine, keeping DVE's stream to rh/zh/m1/hT only. The output history
is accumulated in a 16-step SBUF ring (add on GPSIMD) and DMA-flushed;
initial weight DMAs are ordered by first use with the x tensor split
so only the first 16 steps' slice gates startup.

Per step t (per core):
  PE : r-bias/x/zh-part/m1-part (161 mm) -> c-bias/x (33) ->
       z-bias/x/h (97) -> c-rh (64)
  Act: rs=sigmoid(ps_r) [f16], zs=sigmoid(ps_z) [f32],
       w=sigmoid(-ps_z) [f16], hh=tanh(ps_c) [f16]
  DVE: rh=rs*hT [f16], zh=zs*h_prev [f16], m1=w*hh [f16], hT'=m1+zh
  Pool: h=m1+zh [f32 -> history ring]

All matmul moving operands are f16 (1 cyc/row); weights/x are f16,
gate PSUMs f32. Host assembles [128, T, 8, 8] -> [B, T, U].
"""
import sys

if "/opt/trn_rl_repo" not in sys.path:
    sys.path.insert(0, "/opt/trn_rl_repo")

import numpy as np
from contextlib import ExitStack

import concourse.bass as bass
import concourse.bacc as bacc
import concourse.tile as tile
from concourse import mybir
from concourse.bass_utils import run_bass_kernel_spmd

F32 = mybir.dt.float32
F16 = mybir.dt.float16
MULT = mybir.AluOpType.mult
ADD = mybir.AluOpType.add
SIG = mybir.ActivationFunctionType.Sigmoid
TANH = mybir.ActivationFunctionType.Tanh

N_CORES = 8
B = 64
BS = B // N_CORES  # 8 batch rows per core
D = 512
U = 1024
KC = U // 128      # 8 contraction chunks over U
DC = D // 128      # 4 contraction chunks over D
NUC = U // 128     # 8 output u-chunks
FLUSH = 16         # steps per output-DMA flush


def build(nc, T, reps=1):
    # ---- DRAM I/O (per-core; weights replicated, x sharded) ----
    uz_d = nc.dram_tensor("uz", [128, KC * U], F16, kind="ExternalInput")
    ur_d = nc.dram_tensor("ur", [128, KC * U], F16, kind="ExternalInput")
    uh_d = nc.dram_tensor("uh", [128, KC * U], F16, kind="ExternalInput")
    wz_d = nc.dram_tensor("wz", [128, DC * U], F16, kind="ExternalInput")
    wr_d = nc.dram_tensor("wr", [128, DC * U], F16, kind="ExternalInput")
    wh_d = nc.dram_tensor("wh", [128, DC * U], F16, kind="ExternalInput")
    T0 = min(FLUSH, T)  # steps covered by the early x chunk
    xt0_d = nc.dram_tensor("xt0", [128, DC * T0 * BS], F16, kind="ExternalInput")
    xt1_d = None
    if T > T0:
        xt1_d = nc.dram_tensor("xt1", [128, DC * (T - T0) * BS], F16,
                               kind="ExternalInput")
    b8_d = nc.dram_tensor("b8", [8, 3 * 128], F16, kind="ExternalInput")
    e8_d = nc.dram_tensor("e8", [8, NUC * BS], F16, kind="ExternalInput")
    out_d = nc.dram_tensor("out", [128, T * 64], F32, kind="ExternalOutput")

    with tile.TileContext(nc) as tc, ExitStack() as ctx:
        # DMA order = first-use order for the step-0/1 chain: tiny consts,
        # r-gate weights, the early x chunk, then c/z weights, then bulk x
        const = ctx.enter_context(tc.tile_pool(name="const", bufs=1))
        b8 = const.tile([8, 3 * 128], F16)
        nc.sync.dma_start(b8[:], b8_d[:])
        e8 = const.tile([8, NUC * BS], F16)
        nc.sync.dma_start(e8[:], e8_d[:])
        wr = const.tile([128, DC * U], F16)
        nc.sync.dma_start(wr[:], wr_d[:])
        ur = const.tile([128, KC * U], F16)
        nc.sync.dma_start(ur[:], ur_d[:])
        xt0 = const.tile([128, DC * T0 * BS], F16)
        nc.sync.dma_start(xt0[:], xt0_d[:])
        wh = const.tile([128, DC * U], F16)
        nc.sync.dma_start(wh[:], wh_d[:])
        uh = const.tile([128, KC * U], F16)
        nc.sync.dma_start(uh[:], uh_d[:])
        wz = const.tile([128, DC * U], F16)
        nc.sync.dma_start(wz[:], wz_d[:])
        uz = const.tile([128, KC * U], F16)
        nc.sync.dma_start(uz[:], uz_d[:])
        xt1 = None
        if xt1_d is not None:
            xt1 = const.tile([128, DC * (T - T0) * BS], F16)
            nc.sync.dma_start(xt1[:], xt1_d[:])
        hT0 = const.tile([128, 64], F16)
        nc.vector.memset(hT0[:], 0.0)
        h00 = const.tile([128, 64], F32)
        nc.vector.memset(h00[:], 0.0)

        hpool = ctx.enter_context(tc.tile_pool(name="h", bufs=2))
        sp = ctx.enter_context(tc.tile_pool(name="s", bufs=2))
        histp = ctx.enter_context(tc.tile_pool(name="hist", bufs=2))
        psum = ctx.enter_context(tc.tile_pool(name="ps", bufs=2, space="PSUM"))

        def xgate(ps, wt, g):
            # bias preload (start=True clears the whole [128,64] tile), then
            # W^T x_t: 4 d-chunks x 8 u-chunks of out [128, 8]
            nc.tensor.matmul(ps[:], b8[:, g * 128:(g + 1) * 128], e8[:],
                             start=True, stop=False, skip_group_check=True)
            xtile, tt, tspan = (xt0, t, T0) if t < T0 else (xt1, t - T0, T - T0)
            for dc in range(DC):
                xs = xtile[:, dc * (tspan * BS) + tt * BS:
                           dc * (tspan * BS) + (tt + 1) * BS]
                for uc in range(NUC):
                    nc.tensor.matmul(
                        ps[:, uc * 8:(uc + 1) * 8],
                        wt[:, dc * U + uc * 128: dc * U + (uc + 1) * 128],
                        xs, start=False, stop=False, skip_group_check=True)

        def hgate(ps, ut, mov, stop=True, ks=0, ke=KC):
            # U^T h: k-chunks x 8 u-chunks; stop on the last write of
            # each column slice
            for k in range(ks, ke):
                ms = mov[:, k * 8:(k + 1) * 8]
                for uc in range(NUC):
                    nc.tensor.matmul(
                        ps[:, uc * 8:(uc + 1) * 8],
                        ut[:, k * U + uc * 128: k * U + (uc + 1) * 128],
                        ms, start=False, stop=(stop and k == KC - 1),
                        skip_group_check=True)

        def hgate_ucouter(ps, ut, mov):
            # uc-outer ordering: column slice uc is COMPLETE after its own
            # k-loop, so downstream column-half consumers can start early
            for uc in range(NUC):
                for k in range(KC):
                    nc.tensor.matmul(
                        ps[:, uc * 8:(uc + 1) * 8],
                        ut[:, k * U + uc * 128: k * U + (uc + 1) * 128],
                        mov[:, k * 8:(k + 1) * 8],
                        start=False, stop=(k == KC - 1),
                        skip_group_check=True)

        # r-gate consumes h = m1 + zh as two separate moving tensors, so
        # the critical path runs tanh -> m1 -> r-matmuls without waiting
        # for the materialized hT (which is produced off-path for z/rh).
        m1_prev = hT0
        zh_prev = hT0
        hT_prev = hT0
        hprev = h00[:]
        hist = None
        for rep in range(reps):
          for t in range(T):
            if t % FLUSH == 0:
                hist = histp.tile([128, FLUSH * 64], F32, tag="hist")
            ps_r = psum.tile([128, 64], F32, tag="r")
            ps_z = psum.tile([128, 64], F32, tag="z")
            ps_c = psum.tile([128, 64], F32, tag="c")

            # PE stream: r first (critical path: zh-part pre-runs, m1-part
            # is the only h-dependent leg), then c/z fill the gaps
            xgate(ps_r, wr, 1)
            hgate(ps_r, ur, zh_prev[:], stop=False)
            hgate(ps_r, ur, m1_prev[:])
            xgate(ps_c, wh, 2)

            rs = sp.tile([128, 64], F16, tag="rs")
            nc.scalar.activation(rs[:], ps_r[:], SIG)
            rh = sp.tile([128, 64], F16, tag="rh")
            nc.vector.tensor_mul(rh[:], rs[:], hT_prev[:])

            xgate(ps_z, wz, 0)
            hgate(ps_z, uz, hT_prev[:])
            hgate_ucouter(ps_c, uh, rh[:])

            zs = sp.tile([128, 64], F32, tag="zs")
            nc.scalar.activation(zs[:], ps_z[:], SIG)
            w = sp.tile([128, 64], F16, tag="w")
            nc.scalar.activation(w[:], ps_z[:], SIG, scale=-1.0)
            zh = sp.tile([128, 64], F16, tag="zh")
            nc.vector.tensor_mul(zh[:], zs[:], hprev)

            hh = sp.tile([128, 64], F16, tag="hh")
            nc.scalar.activation(hh[:], ps_c[:], TANH)
            m1 = sp.tile([128, 64], F16, tag="m1")
            nc.vector.tensor_mul(m1[:], w[:], hh[:])
            hT_new = hpool.tile([128, 64], F16, tag="hT")
            nc.vector.tensor_add(hT_new[:], m1[:], zh[:])
            hs = hist[:, (t % FLUSH) * 64: (t % FLUSH + 1) * 64]
            nc.gpsimd.tensor_add(hs, m1[:], zh[:])
            if t % FLUSH == FLUSH - 1 or t == T - 1:
                base = t - (t % FLUSH)
                nc.sync.dma_start(
                    out_d[:, base * 64: (t + 1) * 64],
                    hist[:, : (t % FLUSH + 1) * 64])

            m1_prev = m1
            zh_prev = zh
            hT_prev = hT_new
            hprev = hs

    nc.compile()
    return nc


def _u_layout(M):
    # [U, U] -> [128, KC*U]: out[p, k*U+u] = M[k*128+p, u]
    return np.ascontiguousarray(
        np.asarray(M, np.float32).reshape(KC, 128, U).transpose(1, 0, 2)
        .reshape(128, KC * U)).astype(np.float16)


def _w_layout(M):
    # [D, U] -> [128, DC*U]: out[p, dc*U+u] = M[dc*128+p, u]
    return np.ascontiguousarray(
        np.asarray(M, np.float32).reshape(DC, 128, U).transpose(1, 0, 2)
        .reshape(128, DC * U)).astype(np.float16)


def prepare(inputs, Wz, Uz, bz, Wr, Ur, br, Wh, Uh, bh, T):
    """Build the Bass program and the per-core input maps."""
    x = np.asarray(inputs, dtype=np.float32)[:, :T, :]

    uz, ur, uh = _u_layout(Uz), _u_layout(Ur), _u_layout(Uh)
    wz, wr, wh = _w_layout(Wz), _w_layout(Wr), _w_layout(Wh)
    b8 = np.concatenate(
        [np.asarray(v, np.float32).reshape(8, 128) for v in (bz, br, bh)],
        axis=1).astype(np.float16)
    e8 = np.kron(np.eye(8, dtype=np.float16), np.ones((1, 8), np.float16))
    e8 = np.ascontiguousarray(e8)

    nc = bacc.Bacc("TRN2", target_bir_lowering=False, debug=False,
                   num_devices=N_CORES)
    build(nc, T)

    T0 = min(FLUSH, T)
    in_maps = []
    for c in range(N_CORES):
        xc = x[c * BS:(c + 1) * BS]               # [BS, T, D]
        # xt[p, dc, t, b] = xc[b, t, dc*128+p]
        xtc = xc.reshape(BS, T, DC, 128).transpose(3, 2, 1, 0)  # [p, dc, t, b]
        m = {
            "uz": uz, "ur": ur, "uh": uh, "wz": wz, "wr": wr, "wh": wh,
            "b8": b8, "e8": e8,
            "xt0": np.ascontiguousarray(xtc[:, :, :T0])
                   .reshape(128, DC * T0 * BS).astype(np.float16),
        }
        if T > T0:
            m["xt1"] = np.ascontiguousarray(xtc[:, :, T0:]) \
                .reshape(128, DC * (T - T0) * BS).astype(np.float16)
        in_maps.append(m)
    return nc, in_maps


def assemble(results):
    outs = []
    T = results[0]["out"].shape[1] // 64
    for c in range(N_CORES):
        o = results[c]["out"]                     # [128, T*64]
        # o[p, t*64 + uc*8 + b] = h_t[b, uc*128+p]
        o = o.reshape(128, T, NUC, BS).transpose(3, 1, 2, 0)  # [b, t, uc, p]
        outs.append(np.ascontiguousarray(o.reshape(BS, T, U)))
    return np.concatenate(outs, axis=0)           # [B, T, U]


def kernel(inputs, Wz, Uz, bz, Wr, Ur, br, Wh, Uh, bh, _T=None):
    T = inputs.shape[1] if _T is None else _T
    nc, in_maps = prepare(inputs, Wz, Uz, bz, Wr, Ur, br, Wh, Uh, bh, T)
    res = run_bass_kernel_spmd(nc, in_maps, list(range(N_CORES)))
    return assemble(res.results)



# revision 25
# speedup vs baseline: 7.2286x; 6.7016x over previous
"""Trainium2 Bass kernel for CustomGRU (B=64, T=512, D=512, U=1024).

Sharding: data-parallel over batch across 8 NeuronCores (8 rows each),
weights replicated. Everything runs in the TRANSPOSED (U-major) layout:
the hidden state lives as hT[u, b] tiles [128, (uc, b)] = [128, 64], so
every gate matmul has the WEIGHT chunk [128, 128] as the stationary and
an 8-wide hT / xT chunk as the moving tensor (out [128, 8] PSUM column
slices). The input projection W^T x_t is folded into the recurrence as
4 extra contraction chunks per gate (no phase-1 GEMM, no xz staging in
DRAM, no per-step input DMA), and the bias is preloaded with a single
[K=8] matmul per gate (lhsT = bias reshaped [8, 128], moving =
eye8 kron ones8).

The per-step critical chain is kept to four cross-engine hops:
  sigmoid(ps_r) -> rh = rs*hT (DVE) -> c-rh matmuls (PE) ->
  tanh(ps_c)    -> m1 = (1-z)*hh (DVE) -> next step's r-gate matmuls
by (a) feeding the r-gate h = m1 + zh as TWO separate moving tensors
(the zh part and all bias/x parts pre-run mid-step; hT itself is
materialized off-path for the z-gate / rh), and (b) computing both z
and 1-z as two sigmoids straight off the z PSUM (scale=-1) on the Act
engine, keeping DVE's stream to rh/zh/m1/hT only. The output history
is accumulated in a 16-step SBUF ring (add on GPSIMD) and DMA-flushed;
initial weight DMAs are ordered by first use with the x tensor split
so only the first 16 steps' slice gates startup.

Per step t (per core):
  PE : r-bias/x/zh-part/m1-part (161 mm) -> c-bias/x (33) ->
       z-bias/x/h (97) -> c-rh (64)
  Act: rs=sigmoid(ps_r) [f16], zs=sigmoid(ps_z) [f32],
       w=sigmoid(-ps_z) [f16], hh=tanh(ps_c) [f16]
  DVE: rh=rs*hT [f16], zh=zs*h_prev [f16], m1=w*hh [f16], hT'=m1+zh
  Pool: h=m1+zh [f32 -> history ring]

All matmul moving operands are f16 (1 cyc/row); weights/x are f16,
gate PSUMs f32. Host assembles [128, T, 8, 8] -> [B, T, U].
"""
import sys

if "/opt/trn_rl_repo" not in sys.path:
    sys.path.insert(0, "/opt/trn_rl_repo")

import numpy as np
from contextlib import ExitStack

import concourse.bass as bass
import concourse.bacc as bacc
import concourse.tile as tile
from concourse import mybir
from concourse.bass_utils import run_bass_kernel_spmd

F32 = mybir.dt.float32
F16 = mybir.dt.float16
MULT = mybir.AluOpType.mult
ADD = mybir.AluOpType.add
SIG = mybir.ActivationFunctionType.Sigmoid
TANH = mybir.ActivationFunctionType.Tanh

N_CORES = 8
B = 64
BS = B // N_CORES  # 8 batch rows per core
D = 512
U = 1024
KC = U // 128      # 8 contraction chunks over U
DC = D // 128      # 4 contraction chunks over D
NUC = U // 128     # 8 output u-chunks
FLUSH = 16         # steps per output-DMA flush


def build(nc, T, reps=1):
    # ---- DRAM I/O (per-core; weights replicated, x sharded) ----
    uz_d = nc.dram_tensor("uz", [128, KC * U], F16, kind="ExternalInput")
    ur_d = nc.dram_tensor("ur", [128, KC * U], F16, kind="ExternalInput")
    uh_d = nc.dram_tensor("uh", [128, KC * U], F16, kind="ExternalInput")
    wz_d = nc.dram_tensor("wz", [128, DC * U], F16, kind="ExternalInput")
    wr_d = nc.dram_tensor("wr", [128, DC * U], F16, kind="ExternalInput")
    wh_d = nc.dram_tensor("wh", [128, DC * U], F16, kind="ExternalInput")
    T0 = min(FLUSH, T)  # steps covered by the early x chunk
    xt0_d = nc.dram_tensor("xt0", [128, DC * T0 * BS], F16, kind="ExternalInput")
    xt1_d = None
    if T > T0:
        xt1_d = nc.dram_tensor("xt1", [128, DC * (T - T0) * BS], F16,
                               kind="ExternalInput")
    b8_d = nc.dram_tensor("b8", [8, 3 * 128], F16, kind="ExternalInput")
    e8_d = nc.dram_tensor("e8", [8, NUC * BS], F16, kind="ExternalInput")
    out_d = nc.dram_tensor("out", [128, T * 64], F32, kind="ExternalOutput")

    with tile.TileContext(nc) as tc, ExitStack() as ctx:
        # DMA order = first-use order: tiny consts, the W weights + early x
        # chunk (all that step 0 touches since h0=0), then the U matrices
        # (first needed at t=1), then the bulk of x
        const = ctx.enter_context(tc.tile_pool(name="const", bufs=1))
        b8 = const.tile([8, 3 * 128], F16)
        nc.sync.dma_start(b8[:], b8_d[:])
        e8 = const.tile([8, NUC * BS], F16)
        nc.sync.dma_start(e8[:], e8_d[:])
        wr = const.tile([128, DC * U], F16)
        nc.sync.dma_start(wr[:], wr_d[:])
        xt0 = const.tile([128, DC * T0 * BS], F16)
        nc.sync.dma_start(xt0[:], xt0_d[:])
        wh = const.tile([128, DC * U], F16)
        nc.sync.dma_start(wh[:], wh_d[:])
        wz = const.tile([128, DC * U], F16)
        nc.sync.dma_start(wz[:], wz_d[:])
        ur = const.tile([128, KC * U], F16)
        nc.sync.dma_start(ur[:], ur_d[:])
        uh = const.tile([128, KC * U], F16)
        nc.sync.dma_start(uh[:], uh_d[:])
        uz = const.tile([128, KC * U], F16)
        nc.sync.dma_start(uz[:], uz_d[:])
        xt1 = None
        if xt1_d is not None:
            xt1 = const.tile([128, DC * (T - T0) * BS], F16)
            nc.sync.dma_start(xt1[:], xt1_d[:])
        hT0 = const.tile([128, 64], F16)
        nc.vector.memset(hT0[:], 0.0)
        h00 = const.tile([128, 64], F32)
        nc.vector.memset(h00[:], 0.0)

        hpool = ctx.enter_context(tc.tile_pool(name="h", bufs=2))
        sp = ctx.enter_context(tc.tile_pool(name="s", bufs=2))
        histp = ctx.enter_context(tc.tile_pool(name="hist", bufs=2))
        psum = ctx.enter_context(tc.tile_pool(name="ps", bufs=2, space="PSUM"))

        def xgate(ps, wt, g, stop=False):
            # bias preload (start=True clears the whole [128,64] tile), then
            # W^T x_t: 4 d-chunks x 8 u-chunks of out [128, 8]
            nc.tensor.matmul(ps[:], b8[:, g * 128:(g + 1) * 128], e8[:],
                             start=True, stop=False, skip_group_check=True)
            xtile, tt, tspan = (xt0, t, T0) if t < T0 else (xt1, t - T0, T - T0)
            for dc in range(DC):
                xs = xtile[:, dc * (tspan * BS) + tt * BS:
                           dc * (tspan * BS) + (tt + 1) * BS]
                for uc in range(NUC):
                    nc.tensor.matmul(
                        ps[:, uc * 8:(uc + 1) * 8],
                        wt[:, dc * U + uc * 128: dc * U + (uc + 1) * 128],
                        xs, start=False,
                        stop=(stop and dc == DC - 1),
                        skip_group_check=True)

        def hgate(ps, ut, mov, stop=True, ks=0, ke=KC):
            # U^T h: k-chunks x 8 u-chunks; stop on the last write of
            # each column slice
            for k in range(ks, ke):
                ms = mov[:, k * 8:(k + 1) * 8]
                for uc in range(NUC):
                    nc.tensor.matmul(
                        ps[:, uc * 8:(uc + 1) * 8],
                        ut[:, k * U + uc * 128: k * U + (uc + 1) * 128],
                        ms, start=False, stop=(stop and k == KC - 1),
                        skip_group_check=True)

        def hgate_ucouter(ps, ut, mov):
            # uc-outer ordering: column slice uc is COMPLETE after its own
            # k-loop, so downstream column-half consumers can start early
            for uc in range(NUC):
                for k in range(KC):
                    nc.tensor.matmul(
                        ps[:, uc * 8:(uc + 1) * 8],
                        ut[:, k * U + uc * 128: k * U + (uc + 1) * 128],
                        mov[:, k * 8:(k + 1) * 8],
                        start=False, stop=(k == KC - 1),
                        skip_group_check=True)

        # r-gate consumes h = m1 + zh as two separate moving tensors, so
        # the critical path runs tanh -> m1 -> r-matmuls without waiting
        # for the materialized hT (which is produced off-path for z/rh).
        m1_prev = hT0
        zh_prev = hT0
        hT_prev = hT0
        hprev = h00[:]
        hist = None
        for rep in range(reps):
          for t in range(T):
            if t % FLUSH == 0:
                hist = histp.tile([128, FLUSH * 64], F32, tag="hist")
            ps_r = psum.tile([128, 64], F32, tag="r")
            ps_z = psum.tile([128, 64], F32, tag="z")
            ps_c = psum.tile([128, 64], F32, tag="c")

            # PE stream: r first (critical path: zh-part pre-runs, m1-part
            # is the only h-dependent leg), then c/z fill the gaps.
            # At t=0 h is zero, so ALL h-parts are skipped — the first step
            # needs only the W weights and the early x chunk, letting the
            # recurrence start while the U matrices are still loading.
            xgate(ps_r, wr, 1, stop=(t == 0))
            if t > 0:
                hgate(ps_r, ur, zh_prev[:], stop=False)
                hgate(ps_r, ur, m1_prev[:])
            xgate(ps_c, wh, 2, stop=(t == 0))

            rs = sp.tile([128, 64], F16, tag="rs")
            nc.scalar.activation(rs[:], ps_r[:], SIG)
            rh = sp.tile([128, 64], F16, tag="rh")
            nc.vector.tensor_mul(rh[:], rs[:], hT_prev[:])

            xgate(ps_z, wz, 0, stop=(t == 0))
            if t > 0:
                hgate(ps_z, uz, hT_prev[:])
                hgate_ucouter(ps_c, uh, rh[:])

            zs = sp.tile([128, 64], F32, tag="zs")
            nc.scalar.activation(zs[:], ps_z[:], SIG)
            w = sp.tile([128, 64], F16, tag="w")
            nc.scalar.activation(w[:], ps_z[:], SIG, scale=-1.0)
            zh = sp.tile([128, 64], F16, tag="zh")
            nc.vector.tensor_mul(zh[:], zs[:], hprev)

            hh = sp.tile([128, 64], F16, tag="hh")
            nc.scalar.activation(hh[:], ps_c[:], TANH)
            m1 = sp.tile([128, 64], F16, tag="m1")
            nc.vector.tensor_mul(m1[:], w[:], hh[:])
            hT_new = hpool.tile([128, 64], F16, tag="hT")
            nc.vector.tensor_add(hT_new[:], m1[:], zh[:])
            hs = hist[:, (t % FLUSH) * 64: (t % FLUSH + 1) * 64]
            nc.gpsimd.tensor_add(hs, m1[:], zh[:])
            if t % FLUSH == FLUSH - 1 or t == T - 1:
                base = t - (t % FLUSH)
                nc.sync.dma_start(
                    out_d[:, base * 64: (t + 1) * 64],
                    hist[:, : (t % FLUSH + 1) * 64])

            m1_prev = m1
            zh_prev = zh
            hT_prev = hT_new
            hprev = hs

    nc.compile()
    return nc


def _u_layout(M):
    # [U, U] -> [128, KC*U]: out[p, k*U+u] = M[k*128+p, u]
    return np.ascontiguousarray(
        np.asarray(M, np.float32).reshape(KC, 128, U).transpose(1, 0, 2)
        .reshape(128, KC * U)).astype(np.float16)


def _w_layout(M):
    # [D, U] -> [128, DC*U]: out[p, dc*U+u] = M[dc*128+p, u]
    return np.ascontiguousarray(
        np.asarray(M, np.float32).reshape(DC, 128, U).transpose(1, 0, 2)
        .reshape(128, DC * U)).astype(np.float16)


def prepare(inputs, Wz, Uz, bz, Wr, Ur, br, Wh, Uh, bh, T):
    """Build the Bass program and the per-core input maps."""
    x = np.asarray(inputs, dtype=np.float32)[:, :T, :]

    uz, ur, uh = _u_layout(Uz), _u_layout(Ur), _u_layout(Uh)
    wz, wr, wh = _w_layout(Wz), _w_layout(Wr), _w_layout(Wh)
    b8 = np.concatenate(
        [np.asarray(v, np.float32).reshape(8, 128) for v in (bz, br, bh)],
        axis=1).astype(np.float16)
    e8 = np.kron(np.eye(8, dtype=np.float16), np.ones((1, 8), np.float16))
    e8 = np.ascontiguousarray(e8)

    nc = bacc.Bacc("TRN2", target_bir_lowering=False, debug=False,
                   num_devices=N_CORES)
    build(nc, T)

    T0 = min(FLUSH, T)
    in_maps = []
    for c in range(N_CORES):
        xc = x[c * BS:(c + 1) * BS]               # [BS, T, D]
        # xt[p, dc, t, b] = xc[b, t, dc*128+p]
        xtc = xc.reshape(BS, T, DC, 128).transpose(3, 2, 1, 0)  # [p, dc, t, b]
        m = {
            "uz": uz, "ur": ur, "uh": uh, "wz": wz, "wr": wr, "wh": wh,
            "b8": b8, "e8": e8,
            "xt0": np.ascontiguousarray(xtc[:, :, :T0])
                   .reshape(128, DC * T0 * BS).astype(np.float16),
        }
        if T > T0:
            m["xt1"] = np.ascontiguousarray(xtc[:, :, T0:]) \
                .reshape(128, DC * (T - T0) * BS).astype(np.float16)
        in_maps.append(m)
    return nc, in_maps


def assemble(results):
    outs = []
    T = results[0]["out"].shape[1] // 64
    for c in range(N_CORES):
        o = results[c]["out"]                     # [128, T*64]
        # o[p, t*64 + uc*8 + b] = h_t[b, uc*128+p]
        o = o.reshape(128, T, NUC, BS).transpose(3, 1, 2, 0)  # [b, t, uc, p]
        outs.append(np.ascontiguousarray(o.reshape(BS, T, U)))
    return np.concatenate(outs, axis=0)           # [B, T, U]


def kernel(inputs, Wz, Uz, bz, Wr, Ur, br, Wh, Uh, bh, _T=None):
    T = inputs.shape[1] if _T is None else _T
    nc, in_maps = prepare(inputs, Wz, Uz, bz, Wr, Ur, br, Wh, Uh, bh, T)
    res = run_bass_kernel_spmd(nc, in_maps, list(range(N_CORES)))
    return assemble(res.results)

